# revision 53
# baseline (speedup 1.0000x reference)
"""Trainium2 Bass kernel for nn_Attention_73985106641132.

Multi-head causal attention with RoPE (B=2, S=2048, 16 heads, d=64) + qkv/out
projections, sharded over 8 NeuronCores as batch(2) x head-groups(4).

Per-core device program (all matmuls bf16 into fp32 PSUM):
  - x^T built on-chip via xbar DMA-transposes of a host-precast bf16 copy of x.
  - Q^T,K^T projected directly in transposed layout [d, s] (head-pair chunks of
    128 partitions); V in natural layout [s, d] with an appended ones column so
    the P@V matmul also produces the softmax denominator row (no reductions).
  - RoPE applied in the transposed layout: the head-dim is host-permuted so the
    rotate-half partner lands on the adjacent partition; a 32-lane
    stream_shuffle + two mults + add implement the rotation.
  - scores^T = K^T-tile.T @ Q^T (two heads packed in the 128x128 PE array via
    row tile_position), exp on ScalarE straight out of PSUM (scale fused, no
    max-subtraction: scores are O(1)), causal staircase handled by one bf16
    mask multiply per diagonal tile group.
  - P^T @ [V|1] accumulates numerator + denominator in PSUM; denominator row is
    broadcast across partitions on GpSimd, reciprocal on VectorE, and the
    normalize is fused into the PSUM->SBUF evacuation of the numerator.
  - out_proj contracts K=128 per head-pair chunk from the pair-stacked
    transposed attention output (odd heads written with partition-shifted DVE
    stores); partial results (row-parallel w_out) are summed across head-group
    cores on the host.
  - Program emission order is tuned for Tile's priority scheduler: pair-0
    attention streams over all query chunks while the pair-1 qkv projections
    and RoPE fill TensorE gaps, then the pair-1 sweep with out_proj chained in.
"""

import os
import numpy as np
import ml_dtypes

BF16 = ml_dtypes.bfloat16

S_FULL = 2048
H = 1024
D = 64
NH = 16
HEADS_PER_CORE = 4
N_CORES = 8
HC = H // 128  # hidden contraction chunks


# ---------------------------------------------------------------------------
# host-side helpers
# ---------------------------------------------------------------------------

def _rope_tables(seq_len):
    inv_freq = 1.0 / (10000.0 ** (np.arange(0, D, 2, dtype=np.float32) / D))
    t = np.arange(seq_len, dtype=np.float32)
    freqs = np.einsum("i,j->ij", t, inv_freq)          # [S, 32]
    emb = np.concatenate([freqs, freqs], axis=-1)       # [S, 64]
    return np.cos(emb), np.sin(emb)


def _perm():
    # interleave so rotate-half partners are adjacent: [0,32,1,33,...,31,63]
    p = np.empty(D, dtype=np.int64)
    p[0::2] = np.arange(32)
    p[1::2] = np.arange(32) + 32
    return p


def _host_tables(seq_len):
    cos, sin = _rope_tables(seq_len)  # [S, 64]
    perm = _perm()
    # cos_t rows r (0..127): r2 = r % 64 -> cos[s, perm[r2]]
    cos_t = np.empty((128, seq_len), dtype=np.float32)
    ssin_t = np.empty((128, seq_len), dtype=np.float32)
    for r in range(128):
        r2 = r % 64
        cos_t[r] = cos[:, perm[r2]]
        i = r2 // 2
        if r2 % 2 == 0:
            ssin_t[r] = -sin[:, i]
        else:
            ssin_t[r] = sin[:, i]
    # causal staircase masks: mask_r [128, 1024] = [m_r | m_r] (two head blocks)
    # m_r[k, q] = q >= 128*r + k   for the r-th diagonal k-tile of a q-chunk
    kl = np.arange(128)[:, None]
    ql = np.arange(512)[None, :]
    masks = np.empty((4, 128, 1024), dtype=np.float32)
    for r in range(4):
        m = (ql >= 128 * r + kl).astype(np.float32)
        masks[r, :, 0:512] = m
        masks[r, :, 512:1024] = m
    return cos_t.astype(BF16), ssin_t.astype(BF16), masks.astype(BF16)


def _core_inputs(x, w_qkv, w_out, b, g, seq_len):
    """Build the per-core input map (numpy, host-side sharding + layout prep)."""
    perm = _perm()
    heads = [4 * g + h for h in range(HEADS_PER_CORE)]

    # pre-blocked [HC, S, 128]: each xbar-transpose source is fully contiguous
    x_bf = np.ascontiguousarray(
        x[b, :seq_len].reshape(seq_len, HC, 128).transpose(1, 0, 2)
    ).astype(BF16)

    # wq/wk: [1024, 256]; chunk c (128 cols) = head pair (2c, 2c+1), each head's
    # 64 cols in perm order.
    def qk_slice(block):
        cols = []
        for c in range(2):
            for hi in range(2):
                head = heads[2 * c + hi]
                cols.append(block * H + head * D + perm)
        cols = np.concatenate(cols)
        return np.ascontiguousarray(w_qkv[:, cols]).astype(BF16)

    wq = qk_slice(0)
    wk = qk_slice(1)

    # wv: natural order, head-major
    cols = np.concatenate([2 * H + h * D + np.arange(D) for h in heads])
    wv = np.ascontiguousarray(w_qkv[:, cols]).astype(BF16)  # [1024, 256]

    # wo: [128, 2*1024]: row r, col block pair*H + n
    #   r < 64: head (2*pair), d = r ; r >= 64: head (2*pair+1), d = r - 64
    wo = np.empty((128, 2 * H), dtype=np.float32)
    for pair in range(2):
        for hi in range(2):
            head = heads[2 * pair + hi]
            wo[64 * hi:64 * (hi + 1), pair * H:(pair + 1) * H] = \
                w_out[head * D:(head + 1) * D, :]
    wo = wo.astype(BF16)

    cos_t, ssin_t, masks = _host_tables(seq_len)
    return {
        "x_bf": x_bf, "wq": wq, "wk": wk, "wv": wv, "wo": wo,
        "cos_t": cos_t, "ssin_t": ssin_t, "masks": masks,
    }


# ---------------------------------------------------------------------------
# device program
# ---------------------------------------------------------------------------

def build_program(seq_len=S_FULL):
    import concourse.bass as bass
    import concourse.mybir as mybir
    import concourse.tile as tile
    from concourse import bacc, library_config
    from contextlib import ExitStack

    S = seq_len
    NQC = S // 512          # 512-wide query chunks
    NST = S // 128          # 128-wide seq tiles (k-tiles / V s-tiles)
    bf = mybir.dt.bfloat16
    f32 = mybir.dt.float32
    AF = mybir.ActivationFunctionType
    OP = mybir.AluOpType

    nc = bacc.Bacc("TRN2", target_bir_lowering=False, debug=False)

    x_bf = nc.dram_tensor("x_bf", [HC, S, 128], bf, kind="ExternalInput").ap()
    wq_d = nc.dram_tensor("wq", [H, 256], bf, kind="ExternalInput").ap()
    wk_d = nc.dram_tensor("wk", [H, 256], bf, kind="ExternalInput").ap()
    wv_d = nc.dram_tensor("wv", [H, 256], bf, kind="ExternalInput").ap()
    wo_d = nc.dram_tensor("wo", [128, 2 * H], bf, kind="ExternalInput").ap()
    cos_d = nc.dram_tensor("cos_t", [128, S], bf, kind="ExternalInput").ap()
    ssin_d = nc.dram_tensor("ssin_t", [128, S], bf, kind="ExternalInput").ap()
    masks_d = nc.dram_tensor("masks", [4, 128, 1024], bf, kind="ExternalInput").ap()
    out_d = nc.dram_tensor("out_part", [S, H], f32, kind="ExternalOutput").ap()

    swap_mask = []
    for i in range(16):
        swap_mask += [2 * i + 1, 2 * i]

    with tile.TileContext(nc) as tc, ExitStack() as ctx:
        nc.gpsimd.load_library(library_config.attn)

        const = ctx.enter_context(tc.tile_pool(name="const", bufs=1))
        wq_sb = const.tile([128, HC, 256], bf, name="wq_sb")
        wk_sb = const.tile([128, HC, 256], bf, name="wk_sb")
        wv_sb = const.tile([128, HC, 256], bf, name="wv_sb")
        wo_sb = const.tile([128, 2 * H], bf, name="wo_sb")
        cos_sb = const.tile([128, S], bf, name="cos_sb")
        ssin_sb = const.tile([128, S], bf, name="ssin_sb")
        masks_sb = const.tile([128, 4, 1024], bf, name="masks_sb")

        # wq first (gates the very first matmul), then the x^T transposes
        # (gate everything else), then the remaining weights/tables.
        nc.sync.dma_start(wq_sb[:], wq_d.rearrange("(c p) n -> p c n", p=128))

        # ---- phase A: x^T via xbar transpose --------------------------------
        big = ctx.enter_context(tc.tile_pool(name="big", bufs=1))
        x_T = big.tile([128, HC, S], bf, name="x_T")     # h = di*128 + p
        with nc.named_scope("xT"):
            for di in range(HC):
                nc.sync.dma_start(x_T[:, di, :], x_bf[di], transpose=True)

        nc.sync.dma_start(wk_sb[:], wk_d.rearrange("(c p) n -> p c n", p=128))
        nc.sync.dma_start(wv_sb[:], wv_d.rearrange("(c p) n -> p c n", p=128))
        nc.sync.dma_start(cos_sb[:], cos_d)
        nc.sync.dma_start(ssin_sb[:], ssin_d)
        nc.sync.dma_start(wo_sb[:], wo_d)
        nc.sync.dma_start(masks_sb[:], masks_d.rearrange("r p n -> p r n"))

        # ---- phase B: qkv projections --------------------------------------
        qk_raw = ctx.enter_context(tc.tile_pool(name="qk_raw", bufs=1))
        q_raw = qk_raw.tile([128, 2 * S], bf, name="q_raw")
        k_raw = qk_raw.tile([128, 2 * S], bf, name="k_raw")
        vaug = big.tile([128, NST * (HEADS_PER_CORE * 65)], bf, name="vaug")
        VROW = HEADS_PER_CORE * 65

        # ones columns of vaug
        vaug4 = vaug.rearrange("p (st h e) -> p st h e", st=NST, h=HEADS_PER_CORE)
        nc.gpsimd.memset(vaug4[:, :, :, 64:65], 1.0)

        rot = ctx.enter_context(tc.tile_pool(name="rot", bufs=1))
        q_rot = rot.tile([128, 2 * S], bf, name="q_rot")
        k_rot = rot.tile([128, 2 * S], bf, name="k_rot")

        rtmp = ctx.enter_context(tc.tile_pool(name="rtmp", bufs=2))

        with tc.tile_pool(name="ps_b", bufs=4, space="PSUM") as ps_b:

            def qk_chunk(w_sb, raw, c, pool=None, ptag="psb"):
                with nc.named_scope("qkv"):
                    for ss in range(S // 512):
                        ps = (pool or ps_b).tile([128, 512], f32, tag=ptag)
                        for di in range(HC):
                            nc.tensor.matmul(
                                ps[:],
                                w_sb[:, di, c * 128:(c + 1) * 128],
                                x_T[:, di, ss * 512:(ss + 1) * 512],
                                start=(di == 0), stop=(di == HC - 1),
                            )
                        nc.any.tensor_copy(
                            raw[:, c * S + ss * 512: c * S + (ss + 1) * 512], ps[:])

            def rope_slice(raw, rotd, c, si, sw=512):
                # one 512-col slice of a chunk so downstream scores unblock
                # as early as possible
                with nc.named_scope("rope"):
                    sl = slice(c * S + si * sw, c * S + (si + 1) * sw)
                    tb = slice(si * sw, (si + 1) * sw)
                    sh = rtmp.tile([128, sw], bf, tag="sh")
                    nc.vector.stream_shuffle(sh[:], raw[:, sl], swap_mask)
                    t1 = rtmp.tile([128, sw], bf, tag="t1")
                    nc.vector.tensor_tensor(t1[:], raw[:, sl], cos_sb[:, tb], OP.mult)
                    t2 = rtmp.tile([128, sw], bf, tag="t2")
                    nc.vector.tensor_tensor(t2[:], sh[:], ssin_sb[:, tb], OP.mult)
                    nc.vector.tensor_tensor(rotd[:, sl], t1[:], t2[:], OP.add)

            def rope_chunk(raw, rotd, c):
                for si in range(S // 512):
                    rope_slice(raw, rotd, c, si)

            def v_tiles(sts):
                with nc.named_scope("vproj"):
                    for st in sts:
                        ps = ps_b.tile([128, 256], f32, tag="psb")
                        for di in range(HC):
                            nc.tensor.matmul(
                                ps[:],
                                x_T[:, di, st * 128:(st + 1) * 128],
                                wv_sb[:, di, :],
                                start=(di == 0), stop=(di == HC - 1),
                            )
                        nc.any.tensor_copy(vaug4[:, st, :, 0:64], ps[:])

            qk_chunk(wq_sb, q_raw, 0)
            qk_chunk(wk_sb, k_raw, 0)
            for si in range(S // 512):
                rope_slice(q_raw, q_rot, 0, si)
                rope_slice(k_raw, k_rot, 0, si)
            v_tiles(range(NST))



        # ---- phase D+E: attention + out_proj --------------------------------
        # attn_T [128, 2*S]: head pair `pair` at cols pair*S, head(2p) rows
        # 0-63, head(2p+1) rows 64-127 (partition-shifted normalize writes).
        attn_T = big.tile([128, 2 * S], bf, name="attn_T")

        ps_sc = ctx.enter_context(tc.tile_pool(name="ps_sc", bufs=2, space="PSUM"))
        ps_pv = ctx.enter_context(tc.tile_pool(name="ps_pv", bufs=2, space="PSUM"))
        p_pool = ctx.enter_context(tc.tile_pool(name="p_pool", bufs=12))
        den_pool = ctx.enter_context(tc.tile_pool(name="den", bufs=3))
        zb_pool = ctx.enter_context(tc.tile_pool(name="zb", bufs=3))
        out_pool = ctx.enter_context(tc.tile_pool(name="out_sb", bufs=4))

        def attn_pair(qc, pair):
            with nc.named_scope(f"attn{qc}_{pair}"):
                pv = ps_pv.tile([65, 1024], f32, tag="pv")
                n_kt = 4 * qc + 4
                for t in range(n_kt):
                    # sub-tile = one k-tile, both heads of the pair packed
                    ps = ps_sc.tile([128, 1024], f32, tag="sc")
                    for hi in range(2):
                        nc.tensor.matmul(
                            ps[:, hi * 512:(hi + 1) * 512],
                            k_rot[64 * hi:64 * (hi + 1),
                                  pair * S + t * 128: pair * S + (t + 1) * 128],
                            q_rot[64 * hi:64 * (hi + 1),
                                  pair * S + qc * 512: pair * S + (qc + 1) * 512],
                            start=True, stop=True,
                        )
                    pt = p_pool.tile([128, 1024], bf, tag="pt")
                    r = t - 4 * qc
                    if r <= 0:
                        nc.scalar.activation(pt[:], ps[:], AF.Exp,
                                             scale=float(D) ** -0.5)
                    else:
                        # diagonal sub-tile: cols [0, 128r) of each 512-wide
                        # head block are fully masked -> memset instead of exp
                        w0 = 128 * r
                        pt3 = pt.rearrange("p (b q) -> p b q", b=2)
                        ps3 = ps.rearrange("p (b q) -> p b q", b=2)
                        nc.gpsimd.memset(pt3[:, :, 0:w0], 0.0)
                        nc.scalar.activation(pt3[:, :, w0:512], ps3[:, :, w0:512],
                                             AF.Exp, scale=float(D) ** -0.5)
                    if r >= 0:
                        # mask only the 128-wide staircase band [128r, 128r+128)
                        w0 = 128 * r
                        pt3 = pt.rearrange("p (b q) -> p b q", b=2)
                        mk3 = masks_sb.rearrange("p r (b q) -> p r b q", b=2)
                        nc.vector.tensor_tensor(
                            pt3[:, :, w0:w0 + 128], pt3[:, :, w0:w0 + 128],
                            mk3[:, r, :, w0:w0 + 128], OP.mult)
                    for hi in range(2):
                        h = 2 * pair + hi
                        nc.tensor.matmul(
                            pv[0:65, hi * 512:(hi + 1) * 512],
                            vaug[:, t * VROW + 65 * h: t * VROW + 65 * h + 65],
                            pt[:, hi * 512:(hi + 1) * 512],
                            start=(t == 0), stop=(t == n_kt - 1),
                        )
                den = den_pool.tile([1, 1024], f32, tag="den")
                nc.any.tensor_copy(den[:], pv[64:65, :])
                zb = zb_pool.tile([64, 1024], f32, tag="zb")
                nc.gpsimd.partition_broadcast(zb[:], den[:], channels=64)
                zbr = zb_pool.tile([64, 1024], f32, tag="zbr")
                nc.vector.reciprocal(zbr[:], zb[:])
                for hi in range(2):
                    nc.vector.tensor_tensor(
                        attn_T[64 * hi:64 * (hi + 1),
                               pair * S + qc * 512: pair * S + (qc + 1) * 512],
                        pv[0:64, hi * 512:(hi + 1) * 512],
                        zbr[0:64, hi * 512:(hi + 1) * 512],
                        OP.mult,
                    )

        def oproj(qc):
            with nc.named_scope(f"oproj{qc}"):
                for qt in range(4 * qc, 4 * qc + 4):
                    for nch in range(2):
                        po = ps_pv.tile([128, 512], f32, tag="pv")
                        for pair in range(2):
                            nc.tensor.matmul(
                                po[:],
                                attn_T[:, pair * S + qt * 128: pair * S + (qt + 1) * 128],
                                wo_sb[:, pair * H + nch * 512: pair * H + (nch + 1) * 512],
                                start=(pair == 0), stop=(pair == 1),
                            )
                        ob = out_pool.tile([128, 512], f32, tag="ob")
                        nc.any.tensor_copy(ob[:], po[:])
                        nc.sync.dma_start(
                            out_d[qt * 128:(qt + 1) * 128, nch * 512:(nch + 1) * 512],
                            ob[:])

        attn_pair(0, 0)
        qk_chunk(wq_sb, q_raw, 1, pool=ps_pv, ptag="pv")
        qk_chunk(wk_sb, k_raw, 1, pool=ps_pv, ptag="pv")
        rope_chunk(q_raw, q_rot, 1)
        rope_chunk(k_raw, k_rot, 1)
        for qc in range(NQC - 1, 0, -1):
            attn_pair(qc, 0)
        for qc in range(NQC):
            attn_pair(qc, 1)
            if qc > 0:
                oproj(qc - 1)
        oproj(NQC - 1)

    nc.compile()
    return nc


# ---------------------------------------------------------------------------
# entry point
# ---------------------------------------------------------------------------

_PROGRAM_CACHE = {}


def _get_program(seq_len):
    if seq_len not in _PROGRAM_CACHE:
        _PROGRAM_CACHE[seq_len] = build_program(seq_len)
    return _PROGRAM_CACHE[seq_len]


def kernel(x, w_qkv, w_out):
    from concourse import bass_utils

    x = np.asarray(x, dtype=np.float32)
    w_qkv = np.asarray(w_qkv, dtype=np.float32)
    w_out = np.asarray(w_out, dtype=np.float32)
    B, S, _ = x.shape

    nc = _get_program(S)
    in_maps = []
    for core in range(N_CORES):
        b, g = core // 4, core % 4
        in_maps.append(_core_inputs(x, w_qkv, w_out, b, g, S))

    trace = bool(int(os.environ.get("KERNEL_TRACE", "0")))
    try:
        res = bass_utils.run_bass_kernel_spmd(
            nc, in_maps, core_ids=list(range(N_CORES)), trace=trace,
        )
    except ModuleNotFoundError:
        res = bass_utils.run_bass_kernel_spmd(
            nc, in_maps, core_ids=list(range(N_CORES)), trace=False,
        )
    kernel.last_results = res

    out = np.zeros((B, S, H), dtype=np.float32)
    for core in range(N_CORES):
        b = core // 4
        out[b] += res.results[core]["out_part"]
    return out


# revision 56
# speedup vs baseline: 1.0243x; 1.0243x over previous
"""Trainium2 Bass kernel for nn_Attention_73985106641132.

Multi-head causal attention with RoPE (B=2, S=2048, 16 heads, d=64) + qkv/out
projections, sharded over 8 NeuronCores as batch(2) x head-groups(4).

Per-core device program (all matmuls bf16 into fp32 PSUM):
  - x^T built on-chip via xbar DMA-transposes of a host-precast bf16 copy of x.
  - Q^T,K^T projected directly in transposed layout [d, s] (head-pair chunks of
    128 partitions); V in natural layout [s, d] with an appended ones column so
    the P@V matmul also produces the softmax denominator row (no reductions).
  - RoPE applied in the transposed layout: the head-dim is host-permuted so the
    rotate-half partner lands on the adjacent partition; a 32-lane
    stream_shuffle + two mults + add implement the rotation.
  - scores^T = K^T-tile.T @ Q^T (two heads packed in the 128x128 PE array via
    row tile_position), exp on ScalarE straight out of PSUM (scale fused, no
    max-subtraction: scores are O(1)), causal staircase handled by one bf16
    mask multiply per diagonal tile group.
  - P^T @ [V|1] accumulates numerator + denominator in PSUM; denominator row is
    broadcast across partitions on GpSimd, reciprocal on VectorE, and the
    normalize is fused into the PSUM->SBUF evacuation of the numerator.
  - out_proj contracts K=128 per head-pair chunk from the pair-stacked
    transposed attention output (odd heads written with partition-shifted DVE
    stores); partial results (row-parallel w_out) are summed across head-group
    cores on the host.
  - Program emission order is tuned for Tile's priority scheduler: pair-0
    attention streams over all query chunks while the pair-1 qkv projections
    and RoPE fill TensorE gaps, then the pair-1 sweep with out_proj chained in.
"""

import os
import numpy as np
import ml_dtypes

BF16 = ml_dtypes.bfloat16

S_FULL = 2048
H = 1024
D = 64
NH = 16
HEADS_PER_CORE = 4
N_CORES = 8
HC = H // 128  # hidden contraction chunks


# ---------------------------------------------------------------------------
# host-side helpers
# ---------------------------------------------------------------------------

def _rope_tables(seq_len):
    inv_freq = 1.0 / (10000.0 ** (np.arange(0, D, 2, dtype=np.float32) / D))
    t = np.arange(seq_len, dtype=np.float32)
    freqs = np.einsum("i,j->ij", t, inv_freq)          # [S, 32]
    emb = np.concatenate([freqs, freqs], axis=-1)       # [S, 64]
    return np.cos(emb), np.sin(emb)


def _perm():
    # interleave so rotate-half partners are adjacent: [0,32,1,33,...,31,63]
    p = np.empty(D, dtype=np.int64)
    p[0::2] = np.arange(32)
    p[1::2] = np.arange(32) + 32
    return p


def _host_tables(seq_len):
    cos, sin = _rope_tables(seq_len)  # [S, 64]
    perm = _perm()
    # cos_t rows r (0..127): r2 = r % 64 -> cos[s, perm[r2]]
    cos_t = np.empty((128, seq_len), dtype=np.float32)
    ssin_t = np.empty((128, seq_len), dtype=np.float32)
    for r in range(128):
        r2 = r % 64
        cos_t[r] = cos[:, perm[r2]]
        i = r2 // 2
        if r2 % 2 == 0:
            ssin_t[r] = -sin[:, i]
        else:
            ssin_t[r] = sin[:, i]
    # causal staircase masks: mask_r [128, 1024] = [m_r | m_r] (two head blocks)
    # m_r[k, q] = q >= 128*r + k   for the r-th diagonal k-tile of a q-chunk
    kl = np.arange(128)[:, None]
    ql = np.arange(512)[None, :]
    masks = np.empty((4, 128, 1024), dtype=np.float32)
    for r in range(4):
        m = (ql >= 128 * r + kl).astype(np.float32)
        masks[r, :, 0:512] = m
        masks[r, :, 512:1024] = m
    return cos_t.astype(BF16), ssin_t.astype(BF16), masks.astype(BF16)


def _core_inputs(x, w_qkv, w_out, b, g, seq_len):
    """Build the per-core input map (numpy, host-side sharding + layout prep)."""
    perm = _perm()
    heads = [4 * g + h for h in range(HEADS_PER_CORE)]

    # pre-blocked [HC, S, 128]: each xbar-transpose source is fully contiguous
    x_bf = np.ascontiguousarray(
        x[b, :seq_len].reshape(seq_len, HC, 128).transpose(1, 0, 2)
    ).astype(BF16)

    # wq/wk: [1024, 256]; chunk c (128 cols) = head pair (2c, 2c+1), each head's
    # 64 cols in perm order.
    def qk_slice(block):
        cols = []
        for c in range(2):
            for hi in range(2):
                head = heads[2 * c + hi]
                cols.append(block * H + head * D + perm)
        cols = np.concatenate(cols)
        return np.ascontiguousarray(w_qkv[:, cols]).astype(BF16)

    wq = qk_slice(0)
    wk = qk_slice(1)

    # wv: natural order, head-major
    cols = np.concatenate([2 * H + h * D + np.arange(D) for h in heads])
    wv = np.ascontiguousarray(w_qkv[:, cols]).astype(BF16)  # [1024, 256]

    # wo: [128, 2*1024]: row r, col block pair*H + n
    #   r < 64: head (2*pair), d = r ; r >= 64: head (2*pair+1), d = r - 64
    wo = np.empty((128, 2 * H), dtype=np.float32)
    for pair in range(2):
        for hi in range(2):
            head = heads[2 * pair + hi]
            wo[64 * hi:64 * (hi + 1), pair * H:(pair + 1) * H] = \
                w_out[head * D:(head + 1) * D, :]
    wo = wo.astype(BF16)

    cos_t, ssin_t, masks = _host_tables(seq_len)
    return {
        "x_bf": x_bf, "wq": wq, "wk": wk, "wv": wv, "wo": wo,
        "cos_t": cos_t, "ssin_t": ssin_t, "masks": masks,
    }


# ---------------------------------------------------------------------------
# device program
# ---------------------------------------------------------------------------

def build_program(seq_len=S_FULL):
    import concourse.bass as bass
    import concourse.mybir as mybir
    import concourse.tile as tile
    from concourse import bacc, library_config
    from contextlib import ExitStack

    S = seq_len
    NQC = S // 512          # 512-wide query chunks
    NST = S // 128          # 128-wide seq tiles (k-tiles / V s-tiles)
    bf = mybir.dt.bfloat16
    f32 = mybir.dt.float32
    AF = mybir.ActivationFunctionType
    OP = mybir.AluOpType

    nc = bacc.Bacc("TRN2", target_bir_lowering=False, debug=False)

    x_bf = nc.dram_tensor("x_bf", [HC, S, 128], bf, kind="ExternalInput").ap()
    wq_d = nc.dram_tensor("wq", [H, 256], bf, kind="ExternalInput").ap()
    wk_d = nc.dram_tensor("wk", [H, 256], bf, kind="ExternalInput").ap()
    wv_d = nc.dram_tensor("wv", [H, 256], bf, kind="ExternalInput").ap()
    wo_d = nc.dram_tensor("wo", [128, 2 * H], bf, kind="ExternalInput").ap()
    cos_d = nc.dram_tensor("cos_t", [128, S], bf, kind="ExternalInput").ap()
    ssin_d = nc.dram_tensor("ssin_t", [128, S], bf, kind="ExternalInput").ap()
    masks_d = nc.dram_tensor("masks", [4, 128, 1024], bf, kind="ExternalInput").ap()
    out_d = nc.dram_tensor("out_part", [S, H], f32, kind="ExternalOutput").ap()

    swap_mask = []
    for i in range(16):
        swap_mask += [2 * i + 1, 2 * i]

    with tile.TileContext(nc) as tc, ExitStack() as ctx:
        nc.gpsimd.load_library(library_config.attn)

        const = ctx.enter_context(tc.tile_pool(name="const", bufs=1))
        wq_sb = const.tile([128, HC, 256], bf, name="wq_sb")
        wk_sb = const.tile([128, HC, 256], bf, name="wk_sb")
        wv_sb = const.tile([128, HC, 256], bf, name="wv_sb")
        wo_sb = const.tile([128, 2 * H], bf, name="wo_sb")
        cos_sb = const.tile([128, S], bf, name="cos_sb")
        ssin_sb = const.tile([128, S], bf, name="ssin_sb")
        masks_sb = const.tile([128, 4, 1024], bf, name="masks_sb")

        # wq first (gates the very first matmul), then the x^T transposes
        # (gate everything else), then the remaining weights/tables.
        nc.sync.dma_start(wq_sb[:], wq_d.rearrange("(c p) n -> p c n", p=128))

        # ---- phase A: x^T via xbar transpose --------------------------------
        big = ctx.enter_context(tc.tile_pool(name="big", bufs=1))
        x_T = big.tile([128, HC, S], bf, name="x_T")     # h = di*128 + p
        with nc.named_scope("xT"):
            for di in range(HC):
                nc.sync.dma_start(x_T[:, di, :], x_bf[di], transpose=True)

        nc.sync.dma_start(wk_sb[:], wk_d.rearrange("(c p) n -> p c n", p=128))
        nc.sync.dma_start(wv_sb[:], wv_d.rearrange("(c p) n -> p c n", p=128))
        nc.sync.dma_start(cos_sb[:], cos_d)
        nc.sync.dma_start(ssin_sb[:], ssin_d)
        nc.sync.dma_start(wo_sb[:], wo_d)
        nc.sync.dma_start(masks_sb[:], masks_d.rearrange("r p n -> p r n"))

        # ---- phase B: qkv projections --------------------------------------
        qk_raw = ctx.enter_context(tc.tile_pool(name="qk_raw", bufs=1))
        q_raw = qk_raw.tile([128, 2 * S], bf, name="q_raw")
        k_raw = qk_raw.tile([128, 2 * S], bf, name="k_raw")
        vaug = big.tile([128, NST * (HEADS_PER_CORE * 65)], bf, name="vaug")
        VROW = HEADS_PER_CORE * 65

        # ones columns of vaug
        vaug4 = vaug.rearrange("p (st h e) -> p st h e", st=NST, h=HEADS_PER_CORE)
        nc.gpsimd.memset(vaug4[:, :, :, 64:65], 1.0)

        rot = ctx.enter_context(tc.tile_pool(name="rot", bufs=1))
        q_rot = rot.tile([128, 2 * S], bf, name="q_rot")
        k_rot = rot.tile([128, 2 * S], bf, name="k_rot")

        rtmp = ctx.enter_context(tc.tile_pool(name="rtmp", bufs=2))

        with tc.tile_pool(name="ps_b", bufs=4, space="PSUM") as ps_b:

            def qk_chunk(w_sb, raw, c, pool=None, ptag="psb"):
                with nc.named_scope("qkv"):
                    for ss in range(S // 512):
                        ps = (pool or ps_b).tile([128, 512], f32, tag=ptag)
                        for di in range(HC):
                            nc.tensor.matmul(
                                ps[:],
                                w_sb[:, di, c * 128:(c + 1) * 128],
                                x_T[:, di, ss * 512:(ss + 1) * 512],
                                start=(di == 0), stop=(di == HC - 1),
                            )
                        nc.any.tensor_copy(
                            raw[:, c * S + ss * 512: c * S + (ss + 1) * 512], ps[:])

            def rope_slice(raw, rotd, c, si, sw=512):
                # one 512-col slice of a chunk so downstream scores unblock
                # as early as possible
                with nc.named_scope("rope"):
                    sl = slice(c * S + si * sw, c * S + (si + 1) * sw)
                    tb = slice(si * sw, (si + 1) * sw)
                    sh = rtmp.tile([128, sw], bf, tag="sh")
                    nc.vector.stream_shuffle(sh[:], raw[:, sl], swap_mask)
                    t1 = rtmp.tile([128, sw], bf, tag="t1")
                    nc.vector.tensor_tensor(t1[:], raw[:, sl], cos_sb[:, tb], OP.mult)
                    t2 = rtmp.tile([128, sw], bf, tag="t2")
                    nc.vector.tensor_tensor(t2[:], sh[:], ssin_sb[:, tb], OP.mult)
                    nc.vector.tensor_tensor(rotd[:, sl], t1[:], t2[:], OP.add)

            def rope_chunk(raw, rotd, c):
                for si in range(S // 512):
                    rope_slice(raw, rotd, c, si)

            def v_tiles(sts):
                with nc.named_scope("vproj"):
                    for st in sts:
                        ps = ps_b.tile([128, 256], f32, tag="psb")
                        for di in range(HC):
                            nc.tensor.matmul(
                                ps[:],
                                x_T[:, di, st * 128:(st + 1) * 128],
                                wv_sb[:, di, :],
                                start=(di == 0), stop=(di == HC - 1),
                            )
                        nc.any.tensor_copy(vaug4[:, st, :, 0:64], ps[:])

            qk_chunk(wq_sb, q_raw, 0)
            qk_chunk(wk_sb, k_raw, 0)
            for si in range(S // 512):
                rope_slice(q_raw, q_rot, 0, si)
                rope_slice(k_raw, k_rot, 0, si)
            v_tiles(range(NST))



        # ---- phase D+E: attention + out_proj --------------------------------
        # attn_T [128, 2*S]: head pair `pair` at cols pair*S, head(2p) rows
        # 0-63, head(2p+1) rows 64-127 (partition-shifted normalize writes).
        attn_T = big.tile([128, 2 * S], bf, name="attn_T")

        ps_sc = ctx.enter_context(tc.tile_pool(name="ps_sc", bufs=2, space="PSUM"))
        ps_pv = ctx.enter_context(tc.tile_pool(name="ps_pv", bufs=2, space="PSUM"))
        p_pool = ctx.enter_context(tc.tile_pool(name="p_pool", bufs=12))
        den_pool = ctx.enter_context(tc.tile_pool(name="den", bufs=3))
        zb_pool = ctx.enter_context(tc.tile_pool(name="zb", bufs=3))
        out_pool = ctx.enter_context(tc.tile_pool(name="out_sb", bufs=4))

        def attn_pair(qc, pair):
            with nc.named_scope(f"attn{qc}_{pair}"):
                pv = ps_pv.tile([65, 1024], f32, tag="pv")
                n_kt = 4 * qc + 4
                for t in range(n_kt):
                    # sub-tile = one k-tile, both heads of the pair packed
                    ps = ps_sc.tile([128, 1024], f32, tag="sc")
                    for hi in range(2):
                        nc.tensor.matmul(
                            ps[:, hi * 512:(hi + 1) * 512],
                            k_rot[64 * hi:64 * (hi + 1),
                                  pair * S + t * 128: pair * S + (t + 1) * 128],
                            q_rot[64 * hi:64 * (hi + 1),
                                  pair * S + qc * 512: pair * S + (qc + 1) * 512],
                            start=True, stop=True,
                        )
                    pt = p_pool.tile([128, 1024], bf, tag="pt")
                    r = t - 4 * qc
                    if r <= 0:
                        nc.scalar.activation(pt[:], ps[:], AF.Exp,
                                             scale=float(D) ** -0.5)
                    else:
                        # diagonal sub-tile: cols [0, 128r) of each 512-wide
                        # head block are fully masked -> memset instead of exp
                        w0 = 128 * r
                        pt3 = pt.rearrange("p (b q) -> p b q", b=2)
                        ps3 = ps.rearrange("p (b q) -> p b q", b=2)
                        nc.gpsimd.memset(pt3[:, :, 0:w0], 0.0)
                        nc.scalar.activation(pt3[:, :, w0:512], ps3[:, :, w0:512],
                                             AF.Exp, scale=float(D) ** -0.5)
                    if r >= 0:
                        # mask only the 128-wide staircase band [128r, 128r+128)
                        w0 = 128 * r
                        pt3 = pt.rearrange("p (b q) -> p b q", b=2)
                        mk3 = masks_sb.rearrange("p r (b q) -> p r b q", b=2)
                        nc.vector.tensor_tensor(
                            pt3[:, :, w0:w0 + 128], pt3[:, :, w0:w0 + 128],
                            mk3[:, r, :, w0:w0 + 128], OP.mult)
                    for hi in range(2):
                        h = 2 * pair + hi
                        nc.tensor.matmul(
                            pv[0:65, hi * 512:(hi + 1) * 512],
                            vaug[:, t * VROW + 65 * h: t * VROW + 65 * h + 65],
                            pt[:, hi * 512:(hi + 1) * 512],
                            start=(t == 0), stop=(t == n_kt - 1),
                        )
                # reciprocal straight from the PSUM denominator row, then
                # broadcast the reciprocal across partitions (one stage less
                # than copy->broadcast->reciprocal)
                den = den_pool.tile([1, 1024], f32, tag="den")
                nc.vector.reciprocal(den[:], pv[64:65, :])
                zbr = zb_pool.tile([64, 1024], f32, tag="zbr")
                nc.gpsimd.partition_broadcast(zbr[:], den[:], channels=64)
                for hi in range(2):
                    nc.vector.tensor_tensor(
                        attn_T[64 * hi:64 * (hi + 1),
                               pair * S + qc * 512: pair * S + (qc + 1) * 512],
                        pv[0:64, hi * 512:(hi + 1) * 512],
                        zbr[0:64, hi * 512:(hi + 1) * 512],
                        OP.mult,
                    )

        def oproj(qc):
            with nc.named_scope(f"oproj{qc}"):
                for qt in range(4 * qc, 4 * qc + 4):
                    for nch in range(2):
                        po = ps_pv.tile([128, 512], f32, tag="pv")
                        for pair in range(2):
                            nc.tensor.matmul(
                                po[:],
                                attn_T[:, pair * S + qt * 128: pair * S + (qt + 1) * 128],
                                wo_sb[:, pair * H + nch * 512: pair * H + (nch + 1) * 512],
                                start=(pair == 0), stop=(pair == 1),
                            )
                        ob = out_pool.tile([128, 512], f32, tag="ob")
                        nc.any.tensor_copy(ob[:], po[:])
                        nc.sync.dma_start(
                            out_d[qt * 128:(qt + 1) * 128, nch * 512:(nch + 1) * 512],
                            ob[:])

        attn_pair(0, 0)
        qk_chunk(wq_sb, q_raw, 1, pool=ps_pv, ptag="pv")
        qk_chunk(wk_sb, k_raw, 1, pool=ps_pv, ptag="pv")
        rope_chunk(q_raw, q_rot, 1)
        rope_chunk(k_raw, k_rot, 1)
        for qc in range(NQC - 1, 0, -1):
            attn_pair(qc, 0)
        for qc in range(NQC):
            attn_pair(qc, 1)
            if qc > 0:
                oproj(qc - 1)
        oproj(NQC - 1)

    nc.compile()
    return nc


# ---------------------------------------------------------------------------
# entry point
# ---------------------------------------------------------------------------

_PROGRAM_CACHE = {}


def _get_program(seq_len):
    if seq_len not in _PROGRAM_CACHE:
        _PROGRAM_CACHE[seq_len] = build_program(seq_len)
    return _PROGRAM_CACHE[seq_len]


def kernel(x, w_qkv, w_out):
    from concourse import bass_utils

    x = np.asarray(x, dtype=np.float32)
    w_qkv = np.asarray(w_qkv, dtype=np.float32)
    w_out = np.asarray(w_out, dtype=np.float32)
    B, S, _ = x.shape

    nc = _get_program(S)
    in_maps = []
    for core in range(N_CORES):
        b, g = core // 4, core % 4
        in_maps.append(_core_inputs(x, w_qkv, w_out, b, g, S))

    trace = bool(int(os.environ.get("KERNEL_TRACE", "0")))
    try:
        res = bass_utils.run_bass_kernel_spmd(
            nc, in_maps, core_ids=list(range(N_CORES)), trace=trace,
        )
    except ModuleNotFoundError:
        res = bass_utils.run_bass_kernel_spmd(
            nc, in_maps, core_ids=list(range(N_CORES)), trace=False,
        )
    kernel.last_results = res

    out = np.zeros((B, S, H), dtype=np.float32)
    for core in range(N_CORES):
        b = core // 4
        out[b] += res.results[core]["out_part"]
    return out
